# revision 40
# baseline (speedup 1.0000x reference)
"""BertCrossAttention (relative_key_query) Trainium2 kernel — v5.

Full inputs -> full output. Sharding: 8 cores, core c handles batch b=c//2 and
heads [8*(c%2), 8*(c%2)+8). All sharding/slicing/transposition happens on the
host; each core runs an identical Bass program on its own slices.

Math (per core, per head h):
  q = xq @ Wq^T * (ESCALE/8) + bq*(ESCALE/8)   [Lq=1024, 64]  (f16)
  k = x @ Wk^T + bk                             [Lk=2048, 64]  (f16)
  v = x @ Wv^T                                  [Lk=2048, 64]  (bv added on host)
  S[l,r'] = q.k + q.E[t] + k.(E*ESCALE/8)[t],  t = l + r'
  out = softmax_r(S/ESCALE + mask) @ v  (key axis pre-reversed: r' = 2047-r)

Key layout trick: with the key axis reversed, t = l + r'. QE[l,t]=q[l].E[t]
and KE[r',t]=k[r'].E'[t] are built as dense windowed blocks in DRAM; the
rel-score reads are then plain strided DMAs on flat DRAM.

v5 structure:
  - precision split: the QK path stays f16 end-to-end (fp8 q/k adds ~0.07
    abs score noise -> ~7% softmax distortion, way over budget). Only the
    rel-position tables (small terms, ~0.16 raw) are built from fp8.
  - rel1 (QE term, natural layout [l-part, r'-free]) is read back with
    dma_start_transpose: skew read + 128x128 block transposes fused into 8
    XBAR DMAs per head (QE stored f16; XBAR needs 16-bit). The XBAR is
    shared hw state: concurrent transposes on two queues corrupt both, so
    every transpose stays on the SP queue.
  - QE/KE builds run fp8e4 DoubleRow (2 contraction rows/partition-cycle):
    q/k are recast per head into packed fp8 [32,2,L] by gpsimd casting
    DMAs; E tables are host-packed fp8 (raw-E carries x512 against
    subnormals; the QE evac rescales by 1/512).
  - score assembly: Pool TT (rel1T + r2, SBUF only) -> DVE STT (+ QK psum)
    -> ACT exp (mask rides the exp bias) -> PE PV (bf16). QK for step j+1
    is emitted before the j consumers so PE never idles on the vector
    chain; lh-major score loop bounds the rel1T/r2 working set; v bias is
    added on the host (softmax weights sum to 1).
"""

import sys
from contextlib import ExitStack

import ml_dtypes
import numpy as np

sys.path.insert(0, "/opt/trn_rl_repo")

import concourse.bass as bass
import concourse.mybir as mybir
import concourse.tile as tile
from concourse import bacc
from concourse.masks import make_identity

F32 = mybir.dt.float32
F16 = mybir.dt.float16
BF16 = mybir.dt.bfloat16
F8 = mybir.dt.float8e4
ESCALE = 256.0   # score carry scale (exp divides back out)
EFP8 = 512.0     # extra scale on the fp8 raw-E table (QE evac divides)
DR = mybir.MatmulPerfMode.DoubleRow

B, H, DH, D = 4, 16, 64, 1024
LQ, LK = 1024, 2048
HPC = 8              # heads per core
CH = HPC * DH        # 512 output channels per core
TW = 3072            # E-table columns used (t in [0, 3071))
QW = 2176            # QE block storage width (cols 0..2174 used, 2175 pad)
KW = 1151            # KE block storage width (cols 0..1150 used)
NKT = D // 128       # 8 contraction tiles for projections


def build_nc():
    nc = bacc.Bacc("TRN2", target_bir_lowering=False, debug=False, num_devices=8)

    xqT = nc.dram_tensor("xqT", [D, LQ], F16, kind="ExternalInput")
    xT = nc.dram_tensor("xT", [D, LK], F16, kind="ExternalInput")
    wqT = nc.dram_tensor("wqT", [D, CH], F16, kind="ExternalInput")
    wkT = nc.dram_tensor("wkT", [D, CH], F16, kind="ExternalInput")
    wvT = nc.dram_tensor("wvT", [D, CH], F16, kind="ExternalInput")
    bqv = nc.dram_tensor("bqv", [CH], F32, kind="ExternalInput")
    bkv = nc.dram_tensor("bkv", [CH], F32, kind="ExternalInput")
    ef8d = nc.dram_tensor("ef8d", [32, 2, TW], F8, kind="ExternalInput")
    e8f8d = nc.dram_tensor("e8f8d", [32, 2, TW], F8, kind="ExternalInput")
    maskc = nc.dram_tensor("maskc", [128, 16], F32, kind="ExternalInput")
    out = nc.dram_tensor("out", [LQ, CH], F16, kind="ExternalOutput")

    with tile.TileContext(nc) as tc, ExitStack() as ctx:
        const = ctx.enter_context(tc.tile_pool(name="const", bufs=1))
        ident = const.tile([128, 128], F16)
        make_identity(nc, ident)
        ef8 = const.tile([32, 2, TW], F8, tag="ef8")
        e8f8 = const.tile([32, 2, TW], F8, tag="e8f8")
        nc.sync.dma_start(ef8, ef8d[:, :, :])
        nc.sync.dma_start(e8f8, e8f8d[:, :, :])
        mask_sb = const.tile([128, 16], F32, tag="mask")
        nc.sync.dma_start(mask_sb, maskc[:, :])
        bq_sb = const.tile([128, 4], F32, tag="bq")
        bk_sb = const.tile([128, 4], F32, tag="bk")
        nc.sync.dma_start(bq_sb, bqv.rearrange("(t p) -> p t", p=128))
        nc.sync.dma_start(bk_sb, bkv.rearrange("(t p) -> p t", p=128))

        # persistent per-core activation tensors
        persist = ctx.enter_context(tc.tile_pool(name="persist", bufs=1))
        v_sb = persist.tile([128, 16, HPC, DH + 1], BF16, tag="v")  # [r',j,h,dh|1]
        nc.vector.memset(v_sb[:, :, :, DH], 1.0)
        q_sb, k_sb = [], []
        for m in range(4):
            qm = persist.tile([128, LQ], F16, tag=f"qm{m}")
            q_sb.append(qm)
        for m in range(4):
            km = persist.tile([128, LK], F16, tag=f"km{m}")
            k_sb.append(km)

        # ---------------- Phase 1: projections (f16) ----------------
        proj = ExitStack()
        ppool = proj.enter_context(tc.tile_pool(name="pp", bufs=8, space="PSUM"))
        pact = proj.enter_context(tc.tile_pool(name="pact", bufs=1))
        xq_sb, x_sb, wq_sb, wk_sb, wv_sb = [], [], [], [], []
        for t in range(NKT):
            xt_full = pact.tile([128, LK], F16, tag=f"x{t}")
            nc.sync.dma_start(xt_full[:, 0:LQ], xqT[128 * t:128 * (t + 1), :])
            xq_sb.append(xt_full[:, 0:LQ])
            x_sb.append(xt_full)
            wt = pact.tile([128, CH], F16, tag=f"wq{t}")
            nc.sync.dma_start(wt, wqT[128 * t:128 * (t + 1), :])
            wq_sb.append(wt)
            wt = pact.tile([128, CH], F16, tag=f"wk{t}")
            nc.sync.dma_start(wt, wkT[128 * t:128 * (t + 1), :])
            wk_sb.append(wt)
            wt = pact.tile([128, CH], F16, tag=f"wv{t}")
            nc.sync.dma_start(wt, wvT[128 * t:128 * (t + 1), :])
            wv_sb.append(wt)
        for m in range(4):          # ch tiles of 128
            pss = []
            for n in range(2):
                ps = ppool.tile([128, 512], F32, tag="pp", name="ps")
                pss.append(ps)
            for t in range(NKT):
                for n in range(2):
                    nc.tensor.matmul(
                        pss[n],
                        wq_sb[t][:, 128 * m:128 * (m + 1)],
                        xq_sb[t][:, 512 * n:512 * (n + 1)],
                        start=(t == 0), stop=(t == NKT - 1),
                    )
            for n in range(2):
                nc.scalar.activation(
                    q_sb[m][:, 512 * n:512 * (n + 1)], pss[n],
                    mybir.ActivationFunctionType.Identity,
                    bias=bq_sb[:, m:m + 1],
                )

        # x loads reuse the xq buffers (Q-proj consumed them above)
        for t in range(NKT):
            xt = pact.tile([128, LK], F16, tag=f"x{t}")
            nc.sync.dma_start(xt, xT[128 * t:128 * (t + 1), :])
            x_sb[t] = xt
        for m in range(4):
            pss = []
            for n in range(4):      # r' chunks of 512
                ps = ppool.tile([128, 512], F32, tag="pp", name="ps")
                pss.append(ps)
            for t in range(NKT):
                for n in range(4):
                    nc.tensor.matmul(
                        pss[n],
                        wk_sb[t][:, 128 * m:128 * (m + 1)],
                        x_sb[t][:, 512 * n:512 * (n + 1)],
                        start=(t == 0), stop=(t == NKT - 1),
                    )
            for n in range(4):
                nc.vector.tensor_scalar(
                    k_sb[m][:, 512 * n:512 * (n + 1)], pss[n],
                    bk_sb[:, m:m + 1], None, mybir.AluOpType.add,
                )
        # V projection, natural layout out[r', ch]; bias added on host
        for j in range(16):         # r' tiles of 128
            ps = ppool.tile([128, CH], F32, tag="pp")
            for t in range(NKT):
                nc.tensor.matmul(
                    ps,
                    x_sb[t][:, 128 * j:128 * (j + 1)],
                    wv_sb[t],
                    start=(t == 0), stop=(t == NKT - 1),
                )
            nc.vector.tensor_copy(
                v_sb[:, j, :, 0:DH], ps.rearrange("p (h d) -> p h d", h=HPC)
            )

        # ---------------- phase boundary ----------------
        proj.close()
        with tc.tile_critical():
            nc.all_engine_barrier()

        # ---------------- Phase 2: attention per head ----------------
        qe_dram = ctx.enter_context(tc.tile_pool(name="qed", bufs=2, space="DRAM"))
        ke_dram = ctx.enter_context(tc.tile_pool(name="ked", bufs=2, space="DRAM"))
        qe_stp = ctx.enter_context(tc.tile_pool(name="qest", bufs=1))
        ke_stp = ctx.enter_context(tc.tile_pool(name="kest", bufs=1))
        qf8p = ctx.enter_context(tc.tile_pool(name="qf8p", bufs=2))
        kf8p = ctx.enter_context(tc.tile_pool(name="kf8p", bufs=2))
        rel1p = ctx.enter_context(tc.tile_pool(name="rel1p", bufs=3))
        r2p = ctx.enter_context(tc.tile_pool(name="r2p", bufs=3))
        tmpp = ctx.enter_context(tc.tile_pool(name="tmpp", bufs=3))
        sp = ctx.enter_context(tc.tile_pool(name="sp", bufs=3))
        ptp = ctx.enter_context(tc.tile_pool(name="ptp", bufs=3))
        cnp = ctx.enter_context(tc.tile_pool(name="cnp", bufs=2))
        ctxp = ctx.enter_context(tc.tile_pool(name="ctxp", bufs=2))
        # psum: builds use 2-bank tiles (one evac per 1024 cols), scores
        # use 1-bank tiles (short vector-chain stages); ctx accum gets 1
        bps = ctx.enter_context(tc.tile_pool(name="bps", bufs=2, space="PSUM"))
        sps = ctx.enter_context(tc.tile_pool(name="sps", bufs=3, space="PSUM"))
        cps = ctx.enter_context(tc.tile_pool(name="cps", bufs=1, space="PSUM"))

        def head_slices(h):
            m, base = h // 2, 64 * (h % 2)
            return q_sb[m][base:base + 64, :], k_sb[m][base:base + 64, :]

        def emit_tables(h):
            """Cast q/k to packed fp8, build + store QE (f16) / KE (fp8)
            skew tables; rel1 readback fuses skew + transpose (XBAR on SP),
            r2 is a plain skew read. Returns (rel1t[2], r2[2])."""
            m, base = h // 2, 64 * (h % 2)
            qh, kh = head_slices(h)
            qf8 = qf8p.tile([32, 2, LQ], F8, tag="qf8", name="qf8")
            kf8 = kf8p.tile([32, 2, LK], F8, tag="kf8", name="kf8")
            # cast + partition refold (d -> (d//2, d%2)): gpsimd casting DMA
            nc.gpsimd.dma_start(qf8[:, :, :], qh)
            nc.gpsimd.dma_start(kf8[:, :, :], kh)

            qe_st = qe_stp.tile([128, 8, QW], F16, tag="qe_st", name="qe_st")
            nc.gpsimd.memset(qe_st[:, :, QW - 1], 0.0)  # stored pad col
            ke_st = ke_stp.tile([128, 16, KW], F8, tag="ke_st", name="ke_st")
            qd = qe_dram.tile([8, 128, QW], F16, tag="qe_d", name="qd")
            kd = ke_dram.tile([16, 128, KW], F8, tag="ke_d", name="kd")
            rel1t = [
                rel1p.tile([128, 4, 16, 128], F16, tag="rel1", name=f"rt{lh}")
                for lh in range(2)
            ]
            r2 = [
                r2p.tile([128, 16, 512], F8, tag="r2", name=f"r2{lh}")
                for lh in range(2)
            ]

            def qe_block(i):
                l0 = 128 * i
                # 2-bank psum tiles, one evac per 1024 cols (halved fixed
                # costs; GPSIMD can't read PSUM so evacs split DVE/ACT, and
                # the 1/EFP8 rescale of the fp8-table build rides the evac)
                for ci, (c, w) in enumerate(((0, 1024), (1024, 1024), (2048, 127))):
                    ps = bps.tile([128, 1024], F32, tag="bps", name="qeps")
                    for s0 in range(0, w, 512):
                        sw = min(512, w - s0)
                        nc.tensor.matmul(
                            ps[:, s0:s0 + sw],
                            qf8[:, :, l0:l0 + 128],
                            ef8[:, :, l0 + c + s0:l0 + c + s0 + sw],
                            start=True, stop=True, perf_mode=DR,
                        )
                    if ci == 0:
                        nc.vector.tensor_scalar_mul(
                            qe_st[:, i, c:c + w], ps[:, 0:w], 1.0 / EFP8
                        )
                    else:
                        nc.scalar.activation(
                            qe_st[:, i, c:c + w], ps[:, 0:w],
                            mybir.ActivationFunctionType.Identity,
                            scale=1.0 / EFP8,
                        )
                if i == 3 or i == 7:
                    lh = i // 4
                    dst = bass.AP(
                        tensor=qd.tensor, offset=qd.offset + lh * 4 * 128 * QW,
                        ap=[[QW, 128], [128 * QW, 4], [1, QW]],
                    )
                    nc.sync.dma_start(dst, qe_st[:, 4 * lh:4 * lh + 4, :])
                    # fused skew-read + per-128-block transpose:
                    # rel1t[lh][rr, ii, j, l] = QE[l_g, l_g + 128j + rr].
                    # NOTE: the XBAR is shared hw state — concurrent
                    # dma_start_transpose on two queues corrupts both; keep
                    # every transpose on the SP queue (serialized there).
                    for ii in range(4):
                        src = bass.AP(
                            tensor=qd.tensor,
                            offset=qd.offset + (4 * lh + ii) * 128 * QW,
                            ap=[[QW + 1, 128], [1, LK]],
                        )
                        nc.sync.dma_start_transpose(rel1t[lh][:, ii, :, :], src)

            def ke_block(j):
                r0 = 128 * j
                # attention mask rides the KE evac bias (added once per
                # table element => once per score via the r2 skew read)
                for ci, (c, w) in enumerate(((0, 1024), (1024, 127))):
                    ps = bps.tile([128, 1024], F32, tag="bps", name="keps")
                    for s0 in range(0, w, 512):
                        sw = min(512, w - s0)
                        nc.tensor.matmul(
                            ps[:, s0:s0 + sw],
                            kf8[:, :, r0:r0 + 128],
                            e8f8[:, :, r0 + c + s0:r0 + c + s0 + sw],
                            start=True, stop=True, perf_mode=DR,
                        )
                    if ci == 1 or j % 2 == 1:
                        nc.vector.tensor_scalar(
                            ke_st[:, j, c:c + w], ps[:, 0:w],
                            mask_sb[:, j:j + 1], None, mybir.AluOpType.add,
                        )
                    else:
                        nc.scalar.activation(
                            ke_st[:, j, c:c + w], ps[:, 0:w],
                            mybir.ActivationFunctionType.Identity,
                            bias=mask_sb[:, j:j + 1],
                        )
                if j == 7 or j == 15:
                    j0 = 8 * (j // 8)
                    dst = bass.AP(
                        tensor=kd.tensor, offset=kd.offset + j0 * 128 * KW,
                        ap=[[KW, 128], [128 * KW, 8], [1, KW]],
                    )
                    nc.scalar.dma_start(dst, ke_st[:, j0:j0 + 8, :])
                    # r2 skew-read per stored half so the first score steps
                    # of the next head never wait on the trailing store
                    for lh in range(2):
                        src = bass.AP(
                            tensor=kd.tensor,
                            offset=kd.offset + j0 * 128 * KW + 512 * lh,
                            ap=[[KW + 1, 128], [128 * KW, 8], [1, 512]],
                        )
                        nc.sync.dma_start(
                            out=r2[lh][:, j0:j0 + 8, :], in_=src
                        )

            # de-interleaved: consecutive matmuls keep their stationary
            # operand (one Ldweights per lblk / r'-block). Returned as step
            # closures so the pipeline can weave them between score steps.
            steps = [lambda i=i: qe_block(i) for i in range(8)]
            steps += [lambda j=j: ke_block(j) for j in range(16)]
            return steps, (rel1t, r2)

        def emit_scores(h, rel1t, r2):
            qh, kh = head_slices(h)
            ctx_h = ctxp.tile([128, 8, DH], F16, tag="ctxh", name="ctx_h")
            state = {}

            def begin_lh(lh):
                state["ctx_ps"] = cps.tile(
                    [DH + 1, 512], F32, tag="ctxps", name="ctx_ps"
                )
                state["stage"] = {}

            def qk(lh, j):
                s_ps = sps.tile([128, 512], F32, tag="sps", name="s_ps")
                nc.tensor.matmul(
                    s_ps, kh[:, 128 * j:128 * (j + 1)],
                    qh[:, 512 * lh:512 * (lh + 1)],
                    start=True, stop=False,
                )
                # r2 rides the QK psum via PE (I.T @ r2, fp8 moving). For
                # lh1 the rel1T term also joins on PE (sharing the identity
                # Ldweights) so the vector chain is just the ACT exp; lh0
                # keeps rel1T in the DVE STT — splitting the add cost
                # between PE and DVE balances the engines.
                if lh == 0:
                    nc.tensor.matmul(
                        s_ps, ident, r2[lh][:, j, :], start=False, stop=True,
                    )
                else:
                    nc.tensor.matmul(
                        s_ps, ident, r2[lh][:, j, :], start=False, stop=False,
                    )
                    nc.tensor.matmul(
                        s_ps.rearrange("p (a b) -> p a b", a=4),
                        ident, rel1t[lh][:, :, j, :],
                        start=False, stop=True,
                    )
                return s_ps

            def assemble(lh, j, s_ps):
                if lh == 0:
                    s_sb = sp.tile([128, 512], F16, tag="s_sb", name="s_sb")
                    nc.vector.scalar_tensor_tensor(
                        out=s_sb.rearrange("p (a b) -> p a b", a=4),
                        in0=rel1t[lh][:, :, j, :],
                        scalar=1.0,
                        in1=s_ps.rearrange("p (a b) -> p a b", a=4),
                        op0=mybir.AluOpType.mult, op1=mybir.AluOpType.add,
                    )
                    src_ap = s_sb
                else:
                    src_ap = s_ps
                pt = ptp.tile([128, 512], BF16, tag="pt", name="pt")
                nc.scalar.activation(
                    pt, src_ap, mybir.ActivationFunctionType.Exp,
                    scale=1.0 / ESCALE,
                )
                return pt

            def pv(lh, j, pt):
                nc.tensor.matmul(
                    state["ctx_ps"], v_sb[:, j, h, :], pt,
                    start=(j == 0), stop=(j == 15),
                )

            def score_step(lh, j):
                # depth-2 software pipeline: PE runs QK(j) and PV(j-2)
                # while the vector chain (DVE STT -> ACT exp) drains j-1
                stage = state["stage"]
                if j < 16:
                    stage[j] = [qk(lh, j)]
                if 1 <= j < 17:
                    stage[j - 1].append(assemble(lh, j - 1, stage[j - 1][0]))
                if j >= 2:
                    pv(lh, j - 2, stage.pop(j - 2)[1])

            def end_lh(lh):
                # evacuate ctx + rowsum (f16), transpose 65x128 blocks via a
                # regular matmul cn.T @ I, normalize by 1/rowsum on ACT
                ctx_ps = state["ctx_ps"]
                cn = cnp.tile([DH + 1, 512], F16, tag="ctxn", name="cn")
                nc.vector.tensor_copy(cn, ctx_ps)
                ctt = sps.tile([128, 512], F32, tag="sps", name="ctt")
                for q4 in range(4):
                    i = 4 * lh + q4
                    ct = ctt[:, (DH + 1) * q4:(DH + 1) * (q4 + 1)]
                    nc.tensor.matmul(
                        ct,
                        cn[:, 128 * q4:128 * (q4 + 1)],
                        ident[0:DH + 1, 0:DH + 1],
                        start=True, stop=True,
                    )
                    rs_inv = cnp.tile([128, 1], F32, tag="rsinv", name="rs_inv")
                    nc.vector.reciprocal(rs_inv, ct[:, DH:DH + 1])
                    nc.vector.tensor_scalar(
                        ctx_h[:, i, :], ct[:, 0:DH], rs_inv, None,
                        mybir.AluOpType.mult,
                    )

            def store():
                dst = bass.AP(
                    tensor=out, offset=DH * h,
                    ap=[[CH, 128], [128 * CH, 8], [1, DH]],
                )
                nc.sync.dma_start(dst, ctx_h[:, :, :])

            steps = []
            for lh in range(2):
                steps.append(lambda lh=lh: begin_lh(lh))
                steps += [
                    lambda lh=lh, j=j: score_step(lh, j) for j in range(18)
                ]
                steps.append(lambda lh=lh: end_lh(lh))
            steps.append(store)
            return steps

        # software pipeline: head h's table-build steps are WOVEN between
        # head h-1's score steps so every engine sees both workstreams and
        # psum evac latency hides behind score work
        def weave(a, b):
            out_steps = []
            na, nb = len(a), len(b)
            ia = ib = 0
            while ia < na or ib < nb:
                # pace table steps (a) evenly across score steps (b)
                if ia < na and (ib >= nb or ia * nb <= ib * na):
                    out_steps.append(a[ia]); ia += 1
                else:
                    out_steps.append(b[ib]); ib += 1
            return out_steps

        pending = None
        for h in range(HPC + 1):
            tsteps = []
            if h < HPC:
                tsteps, tabs = emit_tables(h)
            ssteps = []
            if h > 0:
                ssteps = emit_scores(h - 1, *pending)
            merged = []
            nt, ns_ = len(tsteps), len(ssteps)
            Q = 2
            for qi in range(Q):
                merged += tsteps[nt * qi // Q:nt * (qi + 1) // Q]
                merged += ssteps[ns_ * qi // Q:ns_ * (qi + 1) // Q]
            for fn_ in merged:
                fn_()
            if h < HPC:
                pending = tabs

    nc.compile()
    return nc


def make_in_maps(inputs):
    hs = np.asarray(inputs["hidden_states"], np.float32)
    qhs = np.asarray(inputs["query_hidden_states"], np.float32)
    am = np.asarray(inputs["attention_mask"], np.float32)
    Wq = np.asarray(inputs["Wq"], np.float32)
    bq = np.asarray(inputs["bq"], np.float32)
    Wk = np.asarray(inputs["Wk"], np.float32)
    bk = np.asarray(inputs["bk"], np.float32)
    Wv = np.asarray(inputs["Wv"], np.float32)
    de = np.asarray(inputs["dist_emb"], np.float32)

    # q is pre-scaled by ESCALE/8 (via Wq, bq) which covers the QK and q.E
    # terms; the fp8 raw-E table carries an extra EFP8 (QE evac divides);
    # the fp8 k.E' table carries ESCALE/8. exp applies 1/ESCALE + raw mask.
    eT = np.zeros((DH, TW), np.float32)
    eT[:, :3071] = de[:3071].T
    ef8 = (eT * EFP8).reshape(32, 2, TW)        # packed: d = 2p + i
    e8f8 = (eT / 8.0 * ESCALE).reshape(32, 2, TW)

    def f8(x):
        return np.ascontiguousarray(x.astype(ml_dtypes.float8_e4m3fn)).view(np.uint8)

    F16_KEYS = {"xqT", "xT", "wqT", "wkT", "wvT"}
    in_maps = []
    for core in range(8):
        b = core // 2
        hg = core % 2
        sl = slice(CH * hg, CH * (hg + 1))
        m = {
            "xqT": np.ascontiguousarray(qhs[b].T),
            "xT": np.ascontiguousarray(hs[b].T[:, ::-1]),
            "wqT": np.ascontiguousarray(Wq[sl].T) * (ESCALE / 8.0),
            "wkT": np.ascontiguousarray(Wk[sl].T),
            "wvT": np.ascontiguousarray(Wv[sl].T),
            "bqv": np.ascontiguousarray(bq[sl]) * (ESCALE / 8.0),
            "bkv": np.ascontiguousarray(bk[sl]),
            "ef8d": f8(ef8),
            "e8f8d": f8(e8f8),
            "maskc": np.ascontiguousarray(
                am[b, 0, 0, ::-1].reshape(16, 128).T
            ) * ESCALE,
        }
        in_maps.append({
            k: np.ascontiguousarray(
                v.astype(np.float16) if k in F16_KEYS
                else (v if v.dtype == np.uint8 else v.astype(np.float32))
            )
            for k, v in m.items()
        })
    return in_maps


_CACHED = {}


def assemble_output(per_core_results, bv):
    out = np.zeros((B, LQ, D), np.float32)
    for core in range(8):
        b = core // 2
        hg = core % 2
        out[b, :, CH * hg:CH * (hg + 1)] = per_core_results[core]["out"]
    # v bias: sum_r p_r (v_r + bv) = ctx + bv since softmax weights sum to 1
    out += np.asarray(bv, np.float32)[None, None, :]
    return out


def kernel(**inputs):
    from concourse.bass_utils import run_bass_kernel_spmd

    if "nc" not in _CACHED:
        _CACHED["nc"] = build_nc()
    nc = _CACHED["nc"]
    in_maps = make_in_maps(inputs)
    res = run_bass_kernel_spmd(nc, in_maps, list(range(8)))
    _CACHED["last_result"] = res
    return assemble_output(res.results, inputs["bv"])
